# revision 9
# baseline (speedup 1.0000x reference)
"""Trainium2 Bass kernel for a 2-layer hetero GCN (nn_NetGCN).

Math (per relation r with edges (src, dst), weights W, bias b):
    y = relu?( Dk^-1/2 * segsum_dst( (Do^-1/2 * x)[src] ) @ W + b )
Layer 1: y_i + y_b (relations 'interacts' and 'behave', relu inside each).
Layer 2: relation 'interacts' on h, no relu.

Distribution: edges sharded by dst across 8 cores (each core owns a
contiguous 12544-node slice).  Each core gathers source rows from a
replicated node-feature table with `dma_gather` (int16 local indices over
four 25088-row table chunks, spread over 4 SWDGE queues), segment-sums them
into PSUM via one-hot matmuls on TensorE, applies norm/weights/bias, and
the h-table is AllGathered between the layers.
"""

import sys

sys.path.insert(0, "/opt/trn_rl_repo")

import numpy as np

P = 128
NCORES = 8
NCHUNK = 4

_PROGRAM_CACHE = {}


def _host_prep_relation(src, dst, npad, shard, nblk, chunk):
    """Sort one relation's edges by (dst-block, src-chunk); build per-core
    int16 gather indices, per-slot dst-local columns and per-group counts.

    Returns (idx16 [NCORES,128,S//16], dstloc [NCORES,128,S//128],
             counts [NCORES,1,nblk*NCHUNK] uint32, T [nblk,NCHUNK] int)
    """
    blk = dst // P                      # global block id
    chk = src // chunk
    order = np.lexsort((chk, blk))
    src_s = src[order]
    dst_s = dst[order]
    blk_s = blk[order]
    chk_s = chk[order]

    nblk_tot = npad // P
    grp = blk_s * NCHUNK + chk_s        # group id, sorted ascending
    counts = np.bincount(grp, minlength=nblk_tot * NCHUNK).reshape(
        nblk_tot, NCHUNK
    )
    bpc = nblk                          # blocks per core
    counts_c = counts.reshape(NCORES, bpc, NCHUNK)
    T = np.maximum(1, (-(-counts_c // P)).max(axis=0))    # [nblk, NCHUNK]

    slots_per_blk = T.sum(axis=1) * P                     # [nblk]
    S = int(slots_per_blk.sum())                          # slots per core
    # slot offset of each (b, k) group
    off_bk = np.zeros((bpc, NCHUNK), dtype=np.int64)
    pos = 0
    for b in range(bpc):
        for k in range(NCHUNK):
            off_bk[b, k] = pos
            pos += int(T[b, k]) * P

    R = counts_c.max(axis=0)                              # [nblk, NCHUNK]
    idx16 = np.full((NCORES, S), -1, dtype=np.int16)
    dstloc = np.full((NCORES, S), -1.0, dtype=np.float32)
    cnt_arr = np.zeros((NCORES, 1, bpc * NCHUNK), dtype=np.uint32)

    # start of each group's edges in the sorted arrays
    grp_start = np.zeros(nblk_tot * NCHUNK + 1, dtype=np.int64)
    np.cumsum(counts.ravel(), out=grp_start[1:])
    for c in range(NCORES):
        for b in range(bpc):
            gb = c * bpc + b
            for k in range(NCHUNK):
                g = gb * NCHUNK + k
                n = int(counts.ravel()[g])
                e0 = int(grp_start[g])
                s0 = int(off_bk[b, k])
                r = int(R[b, k])
                if n:
                    idx16[c, s0 : s0 + n] = (
                        src_s[e0 : e0 + n] - k * chunk
                    ).astype(np.int16)
                    dstloc[c, s0 : s0 + n] = (
                        dst_s[e0 : e0 + n] - (c * shard + b * P)
                    ).astype(np.float32)
                idx16[c, s0 + n : s0 + r] = 0      # gathered, masked by S
                cnt_arr[c, 0, b * NCHUNK + k] = n

    # wrap: idx j -> [j % 16, j // 16], replicated to all 8 Q7 core groups
    idx_w = np.ascontiguousarray(
        np.tile(idx16.reshape(NCORES, S // 16, 16).transpose(0, 2, 1), (1, 8, 1))
    )
    # dstloc: slot j -> [j % 128, j // 128]
    dst_w = np.ascontiguousarray(
        dstloc.reshape(NCORES, S // P, P).transpose(0, 2, 1)
    )
    return idx_w, dst_w, cnt_arr, T, R


def _build_program(npad, shard, nblk, chunk, s_i, s_b, T_i, T_b, R_i, R_b):
    import concourse.bacc as bacc
    import concourse.tile as tile
    from concourse import library_config, mybir

    f32 = mybir.dt.float32
    i16 = mybir.dt.int16
    u32 = mybir.dt.uint32
    AF = mybir.ActivationFunctionType
    ALU = mybir.AluOpType

    nc = bacc.Bacc(
        "TRN2",
        target_bir_lowering=False,
        debug=False,
        num_devices=NCORES,
        num_swdge_queues=4,
    )

    xn_i = nc.declare_dram_parameter("xn_i", [npad, P], f32, isOutput=False)
    xn_b = nc.declare_dram_parameter("xn_b", [npad, P], f32, isOutput=False)
    idx_i = nc.declare_dram_parameter("idx_i", [P, s_i // 16], i16, isOutput=False)
    idx_b = nc.declare_dram_parameter("idx_b", [P, s_b // 16], i16, isOutput=False)
    dl_i = nc.declare_dram_parameter("dl_i", [P, s_i // P], f32, isOutput=False)
    dl_b = nc.declare_dram_parameter("dl_b", [P, s_b // P], f32, isOutput=False)
    rin_i = nc.declare_dram_parameter("rin_i", [P, nblk], f32, isOutput=False)
    rin_b = nc.declare_dram_parameter("rin_b", [P, nblk], f32, isOutput=False)
    rout2 = nc.declare_dram_parameter("rout2", [P, nblk], f32, isOutput=False)
    w1i = nc.declare_dram_parameter("w1i", [P, P], f32, isOutput=False)
    w1b = nc.declare_dram_parameter("w1b", [P, P], f32, isOutput=False)
    w2 = nc.declare_dram_parameter("w2", [P, P], f32, isOutput=False)
    b1i = nc.declare_dram_parameter("b1i", [1, P], f32, isOutput=False)
    b1b = nc.declare_dram_parameter("b1b", [1, P], f32, isOutput=False)
    b2 = nc.declare_dram_parameter("b2", [1, P], f32, isOutput=False)
    iota_in = nc.declare_dram_parameter("iota", [P, P], f32, isOutput=False)
    ident_in = nc.declare_dram_parameter("ident", [P, P], f32, isOutput=False)
    ones_in = nc.declare_dram_parameter("ones", [1, P], f32, isOutput=False)
    y_out = nc.declare_dram_parameter("y", [shard, P], f32, isOutput=True)

    hn_shard = nc.dram_tensor("hn_shard", [shard, P], f32)
    hn_full = nc.dram_tensor("hn_full", [npad, P], f32, addr_space="Shared")

    with tile.TileContext(nc) as tc:
        nc.gpsimd.load_library(library_config.mlp)
        with (
            tc.tile_pool(name="cst", bufs=1) as cst,
            tc.tile_pool(name="edg", bufs=1) as edg,
            tc.tile_pool(name="gp", bufs=8) as gp,
            tc.tile_pool(name="sp", bufs=4) as sp,
            tc.tile_pool(name="bp", bufs=3) as bp,
            tc.tile_pool(name="pa", bufs=2, space="PSUM") as pa,
            tc.tile_pool(name="pt", bufs=2, space="PSUM") as pt,
            tc.tile_pool(name="py", bufs=2, space="PSUM") as py,
        ):
            def load_cst(t, shape, dtype=f32):
                s = cst.tile(list(shape), dtype, tag=t.name)
                nc.sync.dma_start(out=s[:], in_=t[:])
                return s

            iota_sb = load_cst(iota_in, [P, P])
            ident_sb = load_cst(ident_in, [P, P])
            ones_sb = load_cst(ones_in, [1, P])
            w1i_sb = load_cst(w1i, [P, P])
            w1b_sb = load_cst(w1b, [P, P])
            w2_sb = load_cst(w2, [P, P])
            b1i_sb = load_cst(b1i, [1, P])
            b1b_sb = load_cst(b1b, [1, P])
            b2_sb = load_cst(b2, [1, P])
            rini_sb = load_cst(rin_i, [P, nblk])
            rinb_sb = load_cst(rin_b, [P, nblk])
            rout_sb = load_cst(rout2, [P, nblk])

            h_buf = cst.tile([P, nblk * P], f32, tag="h_buf")


            qctr = [0]

            def load_edges(idx_t, dl_t, s_len, tag):
                idx_sb = edg.tile([P, s_len // 16], i16, tag=f"idx_{tag}")
                nc.sync.dma_start(out=idx_sb[:], in_=idx_t[:])
                dl_sb = edg.tile([P, s_len // P], f32, tag=f"dl_{tag}")
                nc.sync.dma_start(out=dl_sb[:], in_=dl_t[:])
                return idx_sb, dl_sb

            def stage(table_ap, edges, T, R, w_sb, bias_sb, rin_sb, epilogue):
                idx_sb, dl_sb = edges
                off = 0  # slot offset
                for b in range(nblk):
                    agg_ps = pa.tile([P, P], f32, tag="agg")
                    if int(R[b].sum()) == 0:
                        nc.vector.memset(agg_ps[:], 0.0)
                    total = sum(-(-int(R[b, k]) // P) for k in range(NCHUNK))
                    mm = 0
                    for k in range(NCHUNK):
                        tk = int(T[b, k])
                        rk = int(R[b, k])
                        if rk == 0:
                            continue
                        g = gp.tile([P, tk * P], f32, tag="g")
                        nc.gpsimd.dma_gather(
                            out_ap=g[:].rearrange("p (t d) -> p t d", t=tk),
                            in_ap=table_ap[k * chunk : (k + 1) * chunk, :],
                            idxs_ap=idx_sb[:, off // 16 : off // 16 + (-(-rk // 16))],
                            num_idxs=rk,
                            num_idxs_reg=rk,
                            elem_size=P,
                            queue_num=qctr[0] % 4,
                        )
                        qctr[0] += 1
                        s_t = sp.tile([P, tk * P], f32, tag="s")
                        nc.vector.tensor_tensor(
                            out=s_t[:].rearrange("p (t n) -> p t n", t=tk),
                            in0=dl_sb[:, off // P : off // P + tk]
                            .unsqueeze(2)
                            .to_broadcast([P, tk, P]),
                            in1=iota_sb[:].unsqueeze(1).to_broadcast([P, tk, P]),
                            op=ALU.is_equal,
                        )
                        for t in range(-(-rk // P)):
                            rows = min(P, rk - t * P)
                            nc.tensor.matmul(
                                out=agg_ps[:],
                                lhsT=s_t[:rows, t * P : (t + 1) * P],
                                rhs=g[:rows, t * P : (t + 1) * P],
                                start=(mm == 0),
                                stop=(mm == total - 1),
                            )
                            mm += 1
                        off += tk * P
                    # rin * agg  (ACT: copy with per-partition scale)
                    agg_sc = bp.tile([P, P], f32, tag="aggsc")
                    nc.scalar.activation(
                        out=agg_sc[:], in_=agg_ps[:], func=AF.Copy,
                        scale=rin_sb[:, b : b + 1],
                    )
                    trp = pt.tile([P, P], f32, tag="trp")
                    nc.tensor.transpose(out=trp[:], in_=agg_sc[:], identity=ident_sb[:])
                    aggT = bp.tile([P, P], f32, tag="aggT")
                    nc.vector.tensor_copy(out=aggT[:], in_=trp[:])
                    y_ps = py.tile([P, P], f32, tag="yps")
                    nc.tensor.matmul(
                        out=y_ps[:], lhsT=aggT[:], rhs=w_sb[:], start=True, stop=False
                    )
                    nc.tensor.matmul(
                        out=y_ps[:], lhsT=ones_sb[:1, :], rhs=bias_sb[:1, :],
                        start=False, stop=True,
                    )
                    epilogue(b, y_ps)

            def epi_l1i(b, y_ps):
                nc.scalar.activation(
                    out=h_buf[:, b * P : (b + 1) * P], in_=y_ps[:], func=AF.Relu
                )

            def epi_l1b(b, y_ps):
                rb = bp.tile([P, P], f32, tag="rb")
                nc.scalar.activation(out=rb[:], in_=y_ps[:], func=AF.Relu)
                hs = h_buf[:, b * P : (b + 1) * P]
                nc.vector.tensor_tensor(out=hs, in0=hs, in1=rb[:], op=ALU.add)
                hn = bp.tile([P, P], f32, tag="hn")
                nc.scalar.activation(
                    out=hn[:], in_=hs, func=AF.Copy, scale=rout_sb[:, b : b + 1]
                )
                nc.sync.dma_start(out=hn_shard[b * P : (b + 1) * P, :], in_=hn[:])

            def epi_l2(b, y_ps):
                ob = bp.tile([P, P], f32, tag="ob")
                nc.scalar.copy(out=ob[:], in_=y_ps[:])
                nc.sync.dma_start(out=y_out[b * P : (b + 1) * P, :], in_=ob[:])

            edges_i = load_edges(idx_i, dl_i, s_i, "i")
            edges_b = load_edges(idx_b, dl_b, s_b, "b")
            stage(xn_i, edges_i, T_i, R_i, w1i_sb, b1i_sb, rini_sb, epi_l1i)
            stage(xn_b, edges_b, T_b, R_b, w1b_sb, b1b_sb, rinb_sb, epi_l1b)
            nc.gpsimd.collective_compute(
                "AllGather",
                mybir.AluOpType.bypass,
                replica_groups=[list(range(NCORES))],
                ins=[hn_shard[:]],
                outs=[hn_full[:]],
            )
            stage(hn_full, edges_i, T_i, R_i, w2_sb, b2_sb, rini_sb, epi_l2)

    nc.compile()
    return nc


def kernel(x, src_i, dst_i, src_b, dst_b, W1_i, b1_i, W1_b, b1_b, W2, b2):
    from concourse.bass_utils import run_bass_kernel_spmd

    x = np.asarray(x, dtype=np.float32)
    src_i = np.asarray(src_i, dtype=np.int64)
    dst_i = np.asarray(dst_i, dtype=np.int64)
    src_b = np.asarray(src_b, dtype=np.int64)
    dst_b = np.asarray(dst_b, dtype=np.int64)
    W1_i = np.asarray(W1_i, dtype=np.float32)
    b1_1 = np.asarray(b1_i, dtype=np.float32)
    W1_b = np.asarray(W1_b, dtype=np.float32)
    b1_b = np.asarray(b1_b, dtype=np.float32)
    W2 = np.asarray(W2, dtype=np.float32)
    b2 = np.asarray(b2, dtype=np.float32)
    b1_i = b1_1

    n = x.shape[0]
    npad = -(-n // (NCORES * P)) * (NCORES * P)
    shard = npad // NCORES
    nblk = shard // P
    chunk = npad // NCHUNK
    assert chunk <= 32768 and chunk % 16 == 0

    def degs(idx):
        d = np.bincount(idx, minlength=npad).astype(np.float32)
        return np.maximum(d, 1.0) ** -0.5

    ro_i = degs(src_i)
    ri_i = degs(dst_i)
    ro_b = degs(src_b)
    ri_b = degs(dst_b)

    xn_i = np.zeros((npad, P), dtype=np.float32)
    xn_i[:n] = x * ro_i[:n, None]
    xn_b = np.zeros((npad, P), dtype=np.float32)
    xn_b[:n] = x * ro_b[:n, None]

    idx_i, dl_i, cnt_i, T_i, R_i = _host_prep_relation(
        src_i, dst_i, npad, shard, nblk, chunk
    )
    idx_b, dl_b, cnt_b, T_b, R_b = _host_prep_relation(
        src_b, dst_b, npad, shard, nblk, chunk
    )
    s_i = idx_i.shape[2] * 16
    s_b = idx_b.shape[2] * 16

    rin_i = ri_i.reshape(NCORES, nblk, P).transpose(0, 2, 1).copy()
    rin_b = ri_b.reshape(NCORES, nblk, P).transpose(0, 2, 1).copy()
    rout2 = ro_i.reshape(NCORES, nblk, P).transpose(0, 2, 1).copy()

    key = (npad, s_i, s_b, R_i.tobytes(), R_b.tobytes())
    if key not in _PROGRAM_CACHE:
        _PROGRAM_CACHE.clear()
        _PROGRAM_CACHE[key] = _build_program(
            npad, shard, nblk, chunk, s_i, s_b, T_i, T_b, R_i, R_b
        )
    nc = _PROGRAM_CACHE[key]

    iota = np.tile(np.arange(P, dtype=np.float32), (P, 1))
    ident = np.eye(P, dtype=np.float32)
    ones = np.ones((1, P), dtype=np.float32)

    in_maps = []
    for c in range(NCORES):
        in_maps.append(
            {
                "xn_i": xn_i,
                "xn_b": xn_b,
                "idx_i": idx_i[c],
                "idx_b": idx_b[c],
                "dl_i": dl_i[c],
                "dl_b": dl_b[c],
                "rin_i": rin_i[c],
                "rin_b": rin_b[c],
                "rout2": rout2[c],
                "w1i": W1_i,
                "w1b": W1_b,
                "w2": W2,
                "b1i": b1_i.reshape(1, P),
                "b1b": b1_b.reshape(1, P),
                "b2": b2.reshape(1, P),
                "iota": iota,
                "ident": ident,
                "ones": ones,
            }
        )

    import os

    trace = os.environ.get("GCN_TRACE", "0") == "1"
    res = run_bass_kernel_spmd(
        nc, in_maps, core_ids=list(range(NCORES)), trace=trace
    )
    if trace and res.exec_time_ns:
        print(f"HW exec time: {res.exec_time_ns} ns")
    y = np.concatenate([res.results[c]["y"] for c in range(NCORES)], axis=0)
    return y[:n]


# revision 10
# speedup vs baseline: 1.0191x; 1.0191x over previous
"""Trainium2 Bass kernel for a 2-layer hetero GCN (nn_NetGCN).

Math (per relation r with edges (src, dst), weights W, bias b):
    y = relu?( Dk^-1/2 * segsum_dst( (Do^-1/2 * x)[src] ) @ W + b )
Layer 1: y_i + y_b (relations 'interacts' and 'behave', relu inside each).
Layer 2: relation 'interacts' on h, no relu.

Distribution: edges sharded by dst across 8 cores (each core owns a
contiguous 12544-node slice).  Each core gathers source rows from a
replicated node-feature table with `dma_gather` (int16 local indices over
four 25088-row table chunks, spread over 4 SWDGE queues), segment-sums them
into PSUM via one-hot matmuls on TensorE, applies norm/weights/bias, and
the h-table is AllGathered between the layers.
"""

import sys

sys.path.insert(0, "/opt/trn_rl_repo")

import numpy as np

P = 128
NCORES = 8
NCHUNK = 4

_PROGRAM_CACHE = {}


def _host_prep_relation(src, dst, npad, shard, nblk, chunk):
    """Sort one relation's edges by (dst-block, src-chunk); build per-core
    int16 gather indices, per-slot dst-local columns and per-group counts.

    Returns (idx16 [NCORES,128,S//16], dstloc [NCORES,128,S//128],
             counts [NCORES,1,nblk*NCHUNK] uint32, T [nblk,NCHUNK] int)
    """
    blk = dst // P                      # global block id
    chk = src // chunk
    order = np.lexsort((chk, blk))
    src_s = src[order]
    dst_s = dst[order]
    blk_s = blk[order]
    chk_s = chk[order]

    nblk_tot = npad // P
    grp = blk_s * NCHUNK + chk_s        # group id, sorted ascending
    counts = np.bincount(grp, minlength=nblk_tot * NCHUNK).reshape(
        nblk_tot, NCHUNK
    )
    bpc = nblk                          # blocks per core
    counts_c = counts.reshape(NCORES, bpc, NCHUNK)
    T = np.maximum(1, (-(-counts_c // P)).max(axis=0))    # [nblk, NCHUNK]

    slots_per_blk = T.sum(axis=1) * P                     # [nblk]
    S = int(slots_per_blk.sum())                          # slots per core
    # slot offset of each (b, k) group
    off_bk = np.zeros((bpc, NCHUNK), dtype=np.int64)
    pos = 0
    for b in range(bpc):
        for k in range(NCHUNK):
            off_bk[b, k] = pos
            pos += int(T[b, k]) * P

    R = counts_c.max(axis=0)                              # [nblk, NCHUNK]
    idx16 = np.full((NCORES, S), -1, dtype=np.int16)
    dstloc = np.full((NCORES, S), -1.0, dtype=np.float32)
    cnt_arr = np.zeros((NCORES, 1, bpc * NCHUNK), dtype=np.uint32)

    # start of each group's edges in the sorted arrays
    grp_start = np.zeros(nblk_tot * NCHUNK + 1, dtype=np.int64)
    np.cumsum(counts.ravel(), out=grp_start[1:])
    for c in range(NCORES):
        for b in range(bpc):
            gb = c * bpc + b
            for k in range(NCHUNK):
                g = gb * NCHUNK + k
                n = int(counts.ravel()[g])
                e0 = int(grp_start[g])
                s0 = int(off_bk[b, k])
                r = int(R[b, k])
                if n:
                    idx16[c, s0 : s0 + n] = (
                        src_s[e0 : e0 + n] - k * chunk
                    ).astype(np.int16)
                    dstloc[c, s0 : s0 + n] = (
                        dst_s[e0 : e0 + n] - (c * shard + b * P)
                    ).astype(np.float32)
                idx16[c, s0 + n : s0 + r] = 0      # gathered, masked by S
                cnt_arr[c, 0, b * NCHUNK + k] = n

    # wrap: idx j -> [j % 16, j // 16], replicated to all 8 Q7 core groups
    idx_w = np.ascontiguousarray(
        np.tile(idx16.reshape(NCORES, S // 16, 16).transpose(0, 2, 1), (1, 8, 1))
    )
    # dstloc: slot j -> [j % 128, j // 128]
    dst_w = np.ascontiguousarray(
        dstloc.reshape(NCORES, S // P, P).transpose(0, 2, 1)
    )
    return idx_w, dst_w, cnt_arr, T, R


def _build_program(npad, shard, nblk, chunk, s_i, s_b, T_i, T_b, R_i, R_b):
    import concourse.bacc as bacc
    import concourse.tile as tile
    from concourse import library_config, mybir

    f32 = mybir.dt.float32
    i16 = mybir.dt.int16
    u32 = mybir.dt.uint32
    AF = mybir.ActivationFunctionType
    ALU = mybir.AluOpType

    nc = bacc.Bacc(
        "TRN2",
        target_bir_lowering=False,
        debug=False,
        num_devices=NCORES,
        num_swdge_queues=4,
    )

    xn_i = nc.declare_dram_parameter("xn_i", [npad, P], f32, isOutput=False)
    xn_b = nc.declare_dram_parameter("xn_b", [npad, P], f32, isOutput=False)
    idx_i = nc.declare_dram_parameter("idx_i", [P, s_i // 16], i16, isOutput=False)
    idx_b = nc.declare_dram_parameter("idx_b", [P, s_b // 16], i16, isOutput=False)
    dl_i = nc.declare_dram_parameter("dl_i", [P, s_i // P], f32, isOutput=False)
    dl_b = nc.declare_dram_parameter("dl_b", [P, s_b // P], f32, isOutput=False)
    rin_i = nc.declare_dram_parameter("rin_i", [P, nblk], f32, isOutput=False)
    rin_b = nc.declare_dram_parameter("rin_b", [P, nblk], f32, isOutput=False)
    rout2 = nc.declare_dram_parameter("rout2", [P, nblk], f32, isOutput=False)
    w1i = nc.declare_dram_parameter("w1i", [P, P], f32, isOutput=False)
    w1b = nc.declare_dram_parameter("w1b", [P, P], f32, isOutput=False)
    w2 = nc.declare_dram_parameter("w2", [P, P], f32, isOutput=False)
    b1i = nc.declare_dram_parameter("b1i", [1, P], f32, isOutput=False)
    b1b = nc.declare_dram_parameter("b1b", [1, P], f32, isOutput=False)
    b2 = nc.declare_dram_parameter("b2", [1, P], f32, isOutput=False)
    iota_in = nc.declare_dram_parameter("iota", [P, P], f32, isOutput=False)
    ident_in = nc.declare_dram_parameter("ident", [P, P], f32, isOutput=False)
    ones_in = nc.declare_dram_parameter("ones", [1, P], f32, isOutput=False)
    y_out = nc.declare_dram_parameter("y", [shard, P], f32, isOutput=True)

    hn_shard = nc.dram_tensor("hn_shard", [shard, P], f32)
    hn_full = nc.dram_tensor("hn_full", [npad, P], f32, addr_space="Shared")

    with tile.TileContext(nc) as tc:
        nc.gpsimd.load_library(library_config.mlp)
        with (
            tc.tile_pool(name="cst", bufs=1) as cst,
            tc.tile_pool(name="edg", bufs=1) as edg,
            tc.tile_pool(name="gp", bufs=12) as gp,
            tc.tile_pool(name="sp", bufs=6) as sp,
            tc.tile_pool(name="bp", bufs=3) as bp,
            tc.tile_pool(name="pa", bufs=3, space="PSUM") as pa,
            tc.tile_pool(name="pt", bufs=2, space="PSUM") as pt,
            tc.tile_pool(name="py", bufs=3, space="PSUM") as py,
        ):
            def load_cst(t, shape, dtype=f32):
                s = cst.tile(list(shape), dtype, tag=t.name)
                nc.sync.dma_start(out=s[:], in_=t[:])
                return s

            iota_sb = load_cst(iota_in, [P, P])
            ident_sb = load_cst(ident_in, [P, P])
            ones_sb = load_cst(ones_in, [1, P])
            w1i_sb = load_cst(w1i, [P, P])
            w1b_sb = load_cst(w1b, [P, P])
            w2_sb = load_cst(w2, [P, P])
            b1i_sb = load_cst(b1i, [1, P])
            b1b_sb = load_cst(b1b, [1, P])
            b2_sb = load_cst(b2, [1, P])
            rini_sb = load_cst(rin_i, [P, nblk])
            rinb_sb = load_cst(rin_b, [P, nblk])
            rout_sb = load_cst(rout2, [P, nblk])

            h_buf = cst.tile([P, nblk * P], f32, tag="h_buf")


            qctr = [0]

            def load_edges(idx_t, dl_t, s_len, tag):
                idx_sb = edg.tile([P, s_len // 16], i16, tag=f"idx_{tag}")
                nc.sync.dma_start(out=idx_sb[:], in_=idx_t[:])
                dl_sb = edg.tile([P, s_len // P], f32, tag=f"dl_{tag}")
                nc.sync.dma_start(out=dl_sb[:], in_=dl_t[:])
                return idx_sb, dl_sb

            def stage(table_ap, edges, T, R, w_sb, bias_sb, rin_sb, epilogue):
                idx_sb, dl_sb = edges
                off = 0  # slot offset
                for b in range(nblk):
                    agg_ps = pa.tile([P, P], f32, tag="agg")
                    if int(R[b].sum()) == 0:
                        nc.vector.memset(agg_ps[:], 0.0)
                    total = sum(-(-int(R[b, k]) // P) for k in range(NCHUNK))
                    mm = 0
                    for k in range(NCHUNK):
                        tk = int(T[b, k])
                        rk = int(R[b, k])
                        if rk == 0:
                            continue
                        g = gp.tile([P, tk * P], f32, tag="g")
                        nc.gpsimd.dma_gather(
                            out_ap=g[:].rearrange("p (t d) -> p t d", t=tk),
                            in_ap=table_ap[k * chunk : (k + 1) * chunk, :],
                            idxs_ap=idx_sb[:, off // 16 : off // 16 + (-(-rk // 16))],
                            num_idxs=rk,
                            num_idxs_reg=rk,
                            elem_size=P,
                            queue_num=qctr[0] % 4,
                        )
                        qctr[0] += 1
                        s_t = sp.tile([P, tk * P], f32, tag="s")
                        nc.vector.tensor_tensor(
                            out=s_t[:].rearrange("p (t n) -> p t n", t=tk),
                            in0=dl_sb[:, off // P : off // P + tk]
                            .unsqueeze(2)
                            .to_broadcast([P, tk, P]),
                            in1=iota_sb[:].unsqueeze(1).to_broadcast([P, tk, P]),
                            op=ALU.is_equal,
                        )
                        for t in range(-(-rk // P)):
                            rows = min(P, rk - t * P)
                            nc.tensor.matmul(
                                out=agg_ps[:],
                                lhsT=s_t[:rows, t * P : (t + 1) * P],
                                rhs=g[:rows, t * P : (t + 1) * P],
                                start=(mm == 0),
                                stop=(mm == total - 1),
                            )
                            mm += 1
                        off += tk * P
                    # rin * agg  (ACT: copy with per-partition scale)
                    agg_sc = bp.tile([P, P], f32, tag="aggsc")
                    nc.scalar.activation(
                        out=agg_sc[:], in_=agg_ps[:], func=AF.Copy,
                        scale=rin_sb[:, b : b + 1],
                    )
                    trp = pt.tile([P, P], f32, tag="trp")
                    nc.tensor.transpose(out=trp[:], in_=agg_sc[:], identity=ident_sb[:])
                    aggT = bp.tile([P, P], f32, tag="aggT")
                    nc.vector.tensor_copy(out=aggT[:], in_=trp[:])
                    y_ps = py.tile([P, P], f32, tag="yps")
                    nc.tensor.matmul(
                        out=y_ps[:], lhsT=aggT[:], rhs=w_sb[:], start=True, stop=False
                    )
                    nc.tensor.matmul(
                        out=y_ps[:], lhsT=ones_sb[:1, :], rhs=bias_sb[:1, :],
                        start=False, stop=True,
                    )
                    epilogue(b, y_ps)

            def epi_l1i(b, y_ps):
                nc.scalar.activation(
                    out=h_buf[:, b * P : (b + 1) * P], in_=y_ps[:], func=AF.Relu
                )

            def epi_l1b(b, y_ps):
                rb = bp.tile([P, P], f32, tag="rb")
                nc.scalar.activation(out=rb[:], in_=y_ps[:], func=AF.Relu)
                hs = h_buf[:, b * P : (b + 1) * P]
                nc.vector.tensor_tensor(out=hs, in0=hs, in1=rb[:], op=ALU.add)
                hn = bp.tile([P, P], f32, tag="hn")
                nc.scalar.activation(
                    out=hn[:], in_=hs, func=AF.Copy, scale=rout_sb[:, b : b + 1]
                )
                nc.sync.dma_start(out=hn_shard[b * P : (b + 1) * P, :], in_=hn[:])

            def epi_l2(b, y_ps):
                ob = bp.tile([P, P], f32, tag="ob")
                nc.scalar.copy(out=ob[:], in_=y_ps[:])
                nc.sync.dma_start(out=y_out[b * P : (b + 1) * P, :], in_=ob[:])

            edges_i = load_edges(idx_i, dl_i, s_i, "i")
            edges_b = load_edges(idx_b, dl_b, s_b, "b")
            stage(xn_i, edges_i, T_i, R_i, w1i_sb, b1i_sb, rini_sb, epi_l1i)
            stage(xn_b, edges_b, T_b, R_b, w1b_sb, b1b_sb, rinb_sb, epi_l1b)
            nc.gpsimd.collective_compute(
                "AllGather",
                mybir.AluOpType.bypass,
                replica_groups=[list(range(NCORES))],
                ins=[hn_shard[:]],
                outs=[hn_full[:]],
            )
            stage(hn_full, edges_i, T_i, R_i, w2_sb, b2_sb, rini_sb, epi_l2)

    nc.compile()
    return nc


def kernel(x, src_i, dst_i, src_b, dst_b, W1_i, b1_i, W1_b, b1_b, W2, b2):
    from concourse.bass_utils import run_bass_kernel_spmd

    x = np.asarray(x, dtype=np.float32)
    src_i = np.asarray(src_i, dtype=np.int64)
    dst_i = np.asarray(dst_i, dtype=np.int64)
    src_b = np.asarray(src_b, dtype=np.int64)
    dst_b = np.asarray(dst_b, dtype=np.int64)
    W1_i = np.asarray(W1_i, dtype=np.float32)
    b1_1 = np.asarray(b1_i, dtype=np.float32)
    W1_b = np.asarray(W1_b, dtype=np.float32)
    b1_b = np.asarray(b1_b, dtype=np.float32)
    W2 = np.asarray(W2, dtype=np.float32)
    b2 = np.asarray(b2, dtype=np.float32)
    b1_i = b1_1

    n = x.shape[0]
    npad = -(-n // (NCORES * P)) * (NCORES * P)
    shard = npad // NCORES
    nblk = shard // P
    chunk = npad // NCHUNK
    assert chunk <= 32768 and chunk % 16 == 0

    def degs(idx):
        d = np.bincount(idx, minlength=npad).astype(np.float32)
        return np.maximum(d, 1.0) ** -0.5

    ro_i = degs(src_i)
    ri_i = degs(dst_i)
    ro_b = degs(src_b)
    ri_b = degs(dst_b)

    xn_i = np.zeros((npad, P), dtype=np.float32)
    xn_i[:n] = x * ro_i[:n, None]
    xn_b = np.zeros((npad, P), dtype=np.float32)
    xn_b[:n] = x * ro_b[:n, None]

    idx_i, dl_i, cnt_i, T_i, R_i = _host_prep_relation(
        src_i, dst_i, npad, shard, nblk, chunk
    )
    idx_b, dl_b, cnt_b, T_b, R_b = _host_prep_relation(
        src_b, dst_b, npad, shard, nblk, chunk
    )
    s_i = idx_i.shape[2] * 16
    s_b = idx_b.shape[2] * 16

    rin_i = ri_i.reshape(NCORES, nblk, P).transpose(0, 2, 1).copy()
    rin_b = ri_b.reshape(NCORES, nblk, P).transpose(0, 2, 1).copy()
    rout2 = ro_i.reshape(NCORES, nblk, P).transpose(0, 2, 1).copy()

    key = (npad, s_i, s_b, R_i.tobytes(), R_b.tobytes())
    if key not in _PROGRAM_CACHE:
        _PROGRAM_CACHE.clear()
        _PROGRAM_CACHE[key] = _build_program(
            npad, shard, nblk, chunk, s_i, s_b, T_i, T_b, R_i, R_b
        )
    nc = _PROGRAM_CACHE[key]

    iota = np.tile(np.arange(P, dtype=np.float32), (P, 1))
    ident = np.eye(P, dtype=np.float32)
    ones = np.ones((1, P), dtype=np.float32)

    in_maps = []
    for c in range(NCORES):
        in_maps.append(
            {
                "xn_i": xn_i,
                "xn_b": xn_b,
                "idx_i": idx_i[c],
                "idx_b": idx_b[c],
                "dl_i": dl_i[c],
                "dl_b": dl_b[c],
                "rin_i": rin_i[c],
                "rin_b": rin_b[c],
                "rout2": rout2[c],
                "w1i": W1_i,
                "w1b": W1_b,
                "w2": W2,
                "b1i": b1_i.reshape(1, P),
                "b1b": b1_b.reshape(1, P),
                "b2": b2.reshape(1, P),
                "iota": iota,
                "ident": ident,
                "ones": ones,
            }
        )

    import os

    trace = os.environ.get("GCN_TRACE", "0") == "1"
    res = run_bass_kernel_spmd(
        nc, in_maps, core_ids=list(range(NCORES)), trace=trace
    )
    if trace and res.exec_time_ns:
        print(f"HW exec time: {res.exec_time_ns} ns")
    y = np.concatenate([res.results[c]["y"] for c in range(NCORES)], axis=0)
    return y[:n]
